# revision 14
# baseline (speedup 1.0000x reference)
"""Trainium2 Bass kernel for nn_MDCN (mixture-density head forward), v3.

Reference computation (B=2048, F=1024, M=128):
    rho = tanh(feature @ h2rho_w.T + h2rho_b);  rho[:, 0] = 0.95
    pi  = softmax(feature @ h2pi_w.T + h2pi_b)
    var0 = exp(feature @ h2var_w.T + h2var_b)
    var = (1 - exp(rho)) * var0 + 1e-4
    mu  = r*d1 + r*s*d2 + s^2*d3,  s = sqrt(1-r^2),
          d1 = feature@muW, d2 = feature@a, d3 = feature@Z  (a = zstd/wstd*(W-muW))

Design (v3):
  - Data-parallel over batch: 8 cores x 256 rows; weights replicated.
  - All GEMM operands bf16 (PE streams bf16 at 1 col/cycle; fp16 takes 2).
    Features and group-A weights are fused chunk-major in one DRAM tensor so
    a single DMA stream feeds the PE in consumption order.
  - Fused GEMM output columns, split in two groups with separate PSUM tiles:
      A: [-u(128) | pi(128) | d1 d2 d3 pad] (260 cols)  - lands first
      B: [var(128)]                                     - lands last
    The long tanh->s->q->rq->mu chain runs while group B weights stream.
  - rho[:,0]=0.95 clamp folded into the GEMM (zero weights, bias=atanh(0.95)),
    s = sech(u) = (1+tanh u) * exp(-u) exactly, so no epilogue memsets.
  - Epilogue fused across the two 128-row tiles ([128, 2, 128] APs);
    softmax denominators come free via activation accum_out; outputs fp16,
    two store DMAs (pi early, var|mu at the end).
"""

import numpy as np

from contextlib import ExitStack

import concourse.bass as bass
import concourse.bacc as bacc
import concourse.mybir as mybir
import concourse.tile as tile
from concourse.bass_utils import run_bass_kernel_spmd

B, F, M = 2048, 1024, 128
NCORES = 8
BC = B // NCORES            # 256 rows per core
NT = BC // 128              # 2 partition tiles
KC = F // 128               # 8 contraction chunks
HK = KC // 2                # feature-piece split (4|4 keeps the PE chunk-pipelined)
WK = 6                      # var-weight split 6|2: small last piece shortens the B tail
NA = 2 * M + 4              # group-A columns: -u | pi | d1 d2 d3 pad
NB = M                      # group-B columns: var
NF = 2 * 128 + NA           # fused chunk width: ft_t0 | ft_t1 | wa
RHO_1 = 0.95
TAU_INV = 1.0e-4
U0 = float(np.arctanh(np.float64(RHO_1)))   # clamp logit
N_FILLERS = 5
WB_SPLIT = True  # 6|2-chunk var-weight split: small last piece shortens the B tail

F32 = mybir.dt.float32
BF16 = mybir.dt.bfloat16
F16 = mybir.dt.float16
AF = mybir.ActivationFunctionType
OP = mybir.AluOpType
OUT_DT = F16


def _declare_io(nc):
    io = {}
    io["blk"] = nc.dram_tensor("blk", [1, 128 + NA + NB], BF16,
                               kind="ExternalInput").ap()
    io["fw0"] = nc.dram_tensor("fw0", [128, HK, NF], BF16,
                               kind="ExternalInput").ap()
    io["fw1"] = nc.dram_tensor("fw1", [128, KC - HK, NF], BF16,
                               kind="ExternalInput").ap()
    if WB_SPLIT:
        io["wb0"] = nc.dram_tensor("wb0", [128, WK, NB], BF16,
                                   kind="ExternalInput").ap()
        io["wb1"] = nc.dram_tensor("wb1", [128, KC - WK, NB], BF16,
                                   kind="ExternalInput").ap()
    else:
        io["wb"] = nc.dram_tensor("wb", [128, KC, NB], BF16,
                                  kind="ExternalInput").ap()
    # out columns: pi | var | mu
    io["out"] = nc.dram_tensor("out", [NT, 128, 3 * M], OUT_DT,
                               kind="ExternalOutput").ap()
    return io


def _warmup_act(nc, consts):
    # Trigger the ACT exp/tanh table load (~2.7us, once) under the input DMAs.
    warm_in = consts.tile([128, 1], F32, tag="warm_in", name="warm_in")
    warm_out = consts.tile([128, 1], F32, tag="warm_out", name="warm_out")
    nc.vector.memset(warm_in[:], 0.0)
    nc.scalar.activation(warm_out[:], warm_in[:], AF.Exp)


def _filler_srcs(nc, consts):
    # memset on gpsimd: DVE memsets of these cost ~0.8us and sit on the
    # epilogue engine's queue.
    wsrc = consts.tile([1, 128], BF16, tag="pe_w", name="pe_w")
    nc.gpsimd.memset(wsrc[:], 1.0)
    msrc = consts.tile([1, 256], BF16, tag="pe_m", name="pe_m")
    nc.gpsimd.memset(msrc[:], 1.0)
    return wsrc, msrc


def _emit_body(nc, tc, pools, io, n_fillers=N_FILLERS, filler_srcs=None,
               stages=("dma", "mm", "epi")):
    consts, fwpool, psum, work = pools
    do_dma = "dma" in stages
    do_mm = "mm" in stages
    do_epi = "epi" in stages

    # ---- input DMAs ----
    # Tiny bias block on the gpsimd (SWDGE) queue so it does not occupy a
    # slot on the SP ring; big tensors stream on the SP ring in consumption
    # order.  Each extra dma_start costs ~0.5us, so only 4 ring transfers.
    blk = consts.tile([1, 128 + NA + NB], BF16, tag="blk", name="blk")
    fw0 = fwpool.tile([128, HK, NF], BF16, tag="fw0", name="fw0")
    fw1 = fwpool.tile([128, KC - HK, NF], BF16, tag="fw1", name="fw1")
    if WB_SPLIT:
        wb0 = fwpool.tile([128, WK, NB], BF16, tag="wb0", name="wb0")
        wb1 = fwpool.tile([128, KC - WK, NB], BF16, tag="wb1", name="wb1")

        def wb_c(c):
            return wb0[:, c] if c < WK else wb1[:, c - WK]
    else:
        wb = fwpool.tile([128, KC, NB], BF16, tag="wb", name="wb")

        def wb_c(c):
            return wb[:, c]

    if do_dma:
        nc.sync.dma_start(blk[:], io["blk"])
        nc.sync.dma_start(fw0[:], io["fw0"])
        nc.sync.dma_start(fw1[:], io["fw1"])
        if WB_SPLIT:
            nc.sync.dma_start(wb0[:], io["wb0"])
            nc.sync.dma_start(wb1[:], io["wb1"])
        else:
            nc.sync.dma_start(wb[:], io["wb"])

    def fw_c(c):
        return fw0[:, c] if c < HK else fw1[:, c - HK]

    # ---- PSUM: group A (banks 0-1), group B (banks 2-3), scratch (bank 4) --
    PA = psum.tile([128, NT, 512], F32, tag="PA", name="PA")
    PB = psum.tile([128, NT, 512], F32, tag="PB", name="PB")

    # PE warmup fillers: keep the HAM activity window busy during the input
    # DMA phase so real matmuls run at 2.4 GHz.
    if n_fillers:
        wsrc, msrc = filler_srcs or _filler_srcs(nc, consts)
        scratch = psum.tile([128, 256], F32, tag="pe_scratch",
                            name="pe_scratch", bufs=1)
        for _ in range(n_fillers):
            nc.tensor.matmul(scratch[:], wsrc[:], msrc[:], start=True,
                             stop=True)

    # ---- matmuls (chunk-consumption order; bias rows join LAST so the
    # slow-arriving bias block never blocks the PE queue head) ----
    ones = blk[:, 0:128]
    if do_mm:
        # bias rows join right after chunk 0 (the bias block lands early on
        # the ring); the accumulation stop rides the LAST chunk matmuls so
        # the bias never sits on the psum-stop critical path
        for c in range(KC):
            for t in range(NT):
                nc.tensor.matmul(PA[:, t, 0:NA],
                                 fw_c(c)[:, t * 128:(t + 1) * 128],
                                 fw_c(c)[:, 256:256 + NA],
                                 start=(c == 0), stop=(c == KC - 1))
            if c == 0:
                for t in range(NT):
                    nc.tensor.matmul(PA[:, t, 0:NA], ones,
                                     blk[:, 128:128 + NA],
                                     start=False, stop=False)
        for c in range(KC):
            for t in range(NT):
                nc.tensor.matmul(PB[:, t, 0:NB],
                                 fw_c(c)[:, t * 128:(t + 1) * 128],
                                 wb_c(c),
                                 start=(c == 0), stop=(c == KC - 1))
            if c == 0:
                for t in range(NT):
                    nc.tensor.matmul(PB[:, t, 0:NB], ones,
                                     blk[:, 128 + NA:128 + NA + NB],
                                     start=False, stop=False)
    if not do_epi:
        return

    # ---- epilogue (fused across row tiles; per-row scalars from dsb) ----
    def wt(shape, tag, dt=F32):
        return work.tile(shape, dt, tag=tag, name=tag)

    r = wt([128, NT, M], "r")
    E = wt([128, NT, 2 * M], "E")          # exp(-u) | exp(pi logits)
    ssum = wt([128, NT, 1], "ssum")
    rs2 = wt([128, NT], "rs2")
    dsb = wt([128, NT, 4], "dsb")
    s = wt([128, NT, M], "s")
    ss = wt([128, NT, M], "ss")
    q = wt([128, NT, M], "q")
    rq = wt([128, NT, M], "rq")
    erho = wt([128, NT, M], "erho")
    var0 = wt([128, NT, M], "var0")
    vv = wt([128, NT, M], "vv")
    osb = wt([128, NT, 3 * M], "osb", OUT_DT)

    Pu = PA[:, :, 0:M]
    Pd = PA[:, :, 2 * M:2 * M + 4]
    Pv = PB[:, :, 0:NB]
    eneg = E[:, :, 0:M]
    ep = E[:, :, M:2 * M]

    # ACT queue: one fused exp covers e^-u and the softmax numerators
    nc.scalar.activation(r[:], Pu, AF.Tanh, scale=-1.0)
    nc.scalar.activation(E[:], PA[:, :, 0:2 * M], AF.Exp)
    nc.scalar.activation(erho[:], r[:], AF.Exp)

    # DVE queue
    nc.vector.tensor_copy(dsb[:], Pd)
    nc.vector.scalar_tensor_tensor(s[:], r[:], 1.0, eneg,
                                   OP.add, OP.mult)          # s = (1+r)e^-u
    nc.vector.tensor_scalar(q[:, 0], s[:, 0], dsb[:, 0, 1:2],
                            dsb[:, 0, 0:1], OP.mult, OP.add)
    nc.vector.tensor_scalar(q[:, 1], s[:, 1], dsb[:, 1, 1:2],
                            dsb[:, 1, 0:1], OP.mult, OP.add)
    nc.vector.tensor_reduce(ssum[:], ep, mybir.AxisListType.X, OP.add)
    nc.vector.reciprocal(rs2[:], ssum[:, :, 0])
    nc.vector.tensor_mul(ss[:], s[:], s[:])
    nc.vector.tensor_mul(rq[:], r[:], q[:])
    nc.vector.scalar_tensor_tensor(osb[:, 0, 2 * M:3 * M], ss[:, 0],
                                   dsb[:, 0, 2:3], rq[:, 0],
                                   OP.mult, OP.add)          # mu tile0
    nc.vector.scalar_tensor_tensor(osb[:, 1, 2 * M:3 * M], ss[:, 1],
                                   dsb[:, 1, 2:3], rq[:, 1],
                                   OP.mult, OP.add)          # mu tile1

    # pi scaling on ACT (per-row 1/sum as per-partition scale)
    nc.scalar.activation(osb[:, 0, 0:M], ep[:, 0], AF.Copy,
                         scale=rs2[:, 0:1])
    nc.scalar.activation(osb[:, 1, 0:M], ep[:, 1], AF.Copy,
                         scale=rs2[:, 1:2])
    nc.sync.dma_start(io["out"][:, :, 0:M].rearrange("t p j -> p t j"),
                      osb[:, :, 0:M])                        # pi out

    # var path (group B)
    nc.scalar.activation(var0[:], Pv, AF.Exp)
    nc.vector.tensor_mul(vv[:], erho[:], var0[:])
    nc.vector.scalar_tensor_tensor(osb[:, :, M:2 * M], var0[:], TAU_INV,
                                   vv[:], OP.add, OP.subtract)  # var
    nc.scalar.dma_start(io["out"][:, :, M:3 * M].rearrange("t p j -> p t j"),
                        osb[:, :, M:3 * M])                    # var|mu out


def _pools(tc, ctx):
    consts = ctx.enter_context(tc.tile_pool(name="consts", bufs=1))
    fwpool = ctx.enter_context(tc.tile_pool(name="fw", bufs=1))
    psum = ctx.enter_context(tc.tile_pool(name="psum", bufs=1, space="PSUM"))
    work = ctx.enter_context(tc.tile_pool(name="work", bufs=1))
    return consts, fwpool, psum, work


def _build_nc():
    nc = bacc.Bacc("TRN2", target_bir_lowering=False, debug=False)
    io = _declare_io(nc)
    with tile.TileContext(nc) as tc, ExitStack() as ctx:
        pools = _pools(tc, ctx)
        _warmup_act(nc, pools[0])
        fsrcs = _filler_srcs(nc, pools[0])
        _emit_body(nc, tc, pools, io, filler_srcs=fsrcs)
    nc.compile()
    return nc


def build_loop_nc(reps, n_fillers=N_FILLERS):
    """Timing variant: body repeated inside one NEFF (full-barrier
    back-edge => per-iter span ~ single-shot time)."""
    nc = bacc.Bacc("TRN2", target_bir_lowering=False, debug=False)
    io = _declare_io(nc)
    with tile.TileContext(nc) as tc, ExitStack() as ctx:
        pools = _pools(tc, ctx)
        _warmup_act(nc, pools[0])
        fsrcs = _filler_srcs(nc, pools[0])
        with tc.For_i(0, reps, 1):
            _emit_body(nc, tc, pools, io, n_fillers=n_fillers,
                       filler_srcs=fsrcs)
    nc.compile()
    return nc


_CACHE = {}


def _get_nc():
    if "nc" not in _CACHE:
        _CACHE["nc"] = _build_nc()
    return _CACHE["nc"]


def _host_prep(inputs):
    f32 = np.float32
    import ml_dtypes
    bf16 = ml_dtypes.bfloat16

    feature = np.ascontiguousarray(inputs["feature"], dtype=f32)
    muW = np.asarray(inputs["muW"], dtype=f32)
    W = np.asarray(inputs["W"], dtype=f32)
    Z = np.asarray(inputs["Z"], dtype=f32)
    logvarW = np.asarray(inputs["logvarW"], dtype=f32)
    logvarZ = np.asarray(inputs["logvarZ"], dtype=f32)

    wstd = np.sqrt(np.exp(logvarW)).astype(f32)
    zstd = np.sqrt(np.exp(logvarZ)).astype(f32)
    a = ((zstd / wstd).astype(f32) * (W - muW)).astype(f32)
    dcols = np.stack([muW, a, Z, np.zeros_like(muW)], axis=1)  # [F, 4]

    wrho_neg = -np.asarray(inputs["h2rho_w"], dtype=f32).T     # [F, M] (-u)
    wrho_neg[:, 0] = 0.0                                       # clamp col
    wpi = np.asarray(inputs["h2pi_w"], dtype=f32).T
    wvar = np.asarray(inputs["h2var_w"], dtype=f32).T

    wa = np.concatenate([wrho_neg, wpi, dcols], axis=1)        # [F, NA]
    wa = wa.reshape(KC, 128, NA)                               # [c, p, NA]
    wb = wvar.reshape(KC, 128, NB).transpose(1, 0, 2)          # [p, c, NB]

    brho = -np.asarray(inputs["h2rho_b"], dtype=f32)
    brho[0] = -U0                                              # -atanh(0.95)
    blk = np.concatenate(
        [np.ones(128, dtype=f32), brho,
         np.asarray(inputs["h2pi_b"], dtype=f32),
         np.zeros(4, dtype=f32),
         np.asarray(inputs["h2var_b"], dtype=f32)]
    ).reshape(1, 128 + NA + NB)

    blk_ = np.ascontiguousarray(blk, dtype=bf16)
    if WB_SPLIT:
        wmap = {"wb0": np.ascontiguousarray(wb[:, 0:WK], dtype=bf16),
                "wb1": np.ascontiguousarray(wb[:, WK:KC], dtype=bf16)}
    else:
        wmap = {"wb": np.ascontiguousarray(wb, dtype=bf16)}

    in_maps = []
    for cidx in range(NCORES):
        shard = feature[cidx * BC:(cidx + 1) * BC]             # [BC, F]
        featT = shard.T.reshape(KC, 128, NT * 128)             # [c, p, t*j]
        fw = np.concatenate([featT, wa], axis=2)               # [c, p, NF]
        fw = fw.transpose(1, 0, 2)                             # [p, c, NF]
        fw0 = np.ascontiguousarray(fw[:, 0:HK], dtype=bf16)
        fw1 = np.ascontiguousarray(fw[:, HK:KC], dtype=bf16)
        in_maps.append({"blk": blk_, "fw0": fw0, "fw1": fw1, **wmap})
    return in_maps


def kernel(**inputs):
    nc = _get_nc()
    in_maps = _host_prep(inputs)
    res = run_bass_kernel_spmd(nc, in_maps, list(range(NCORES)))
    full = np.concatenate(
        [np.asarray(res.results[c]["out"]).reshape(BC, 3 * M)
         for c in range(NCORES)], axis=0).astype(np.float32)
    pi = np.ascontiguousarray(full[:, 0:M])
    var = np.ascontiguousarray(full[:, M:2 * M])
    mu = np.ascontiguousarray(full[:, 2 * M:3 * M])
    return pi, mu, var
